# revision 67
# baseline (speedup 1.0000x reference)
"""Trainium2 Bass kernel for nn_Attention_56822417326562 (dense transformer block).

Sharding: data-parallel over batch — core i computes batch element i entirely
(B=8 over 8 NeuronCores, no collectives).

Per-core math (x: [512, 1600]):
  BN folded into weights on host; softmax scale folded into q.
  The whole attention path runs in fp8e4m3 with DoubleRow perf mode (2 fp8
  contraction rows per PE cell per cycle); safe because the attention output
  is ~7x smaller than the depthwise-conv branch it is summed with:
    - q/k/E/vT stored as [p, 2, n] fp8; scores pad the pair slot with zeros
      (contraction kd=32), attn-out pairs adjacent m-tiles (2a, 2a+1)
    - scores computed TRANSPOSED (S^T[m,n]=k.q) via PE row tiling
      (tile_position=(32i,0)), exp on ScalarE with a -4 bias folded in so
      exp(S-4) fits fp8 range (softmax-invariant)
    - out_un[d,n] and denominator s[n] in one PE accumulation via a ones
      column in v^T
  pe = depthwise 3x3: image rows 0-19 as 9 diagonal bf16 matmuls (PE filler
  that keeps the systolic array hot), rows 20-39 on DVE scalar_tensor_tensor.
  1/s applied via 1-term bf16 ones-matmul broadcast through PSUM.
  y = WpT.T @ ((out_un * (1/s)) + pe) + bias; bias added in the DVE
  PSUM->SBUF drain, output DMA'd as bf16.
  Work is scheduled as filler "jobs" inside the exp-bound attention slots
  (stage-A tail, v-natural, dwconv, prior-half projection) to keep the PE
  continuously busy without long dependency stalls.
"""
import sys

sys.path.insert(0, "/opt/trn_rl_repo")

import numpy as np

DIM = 512
NH = 8
HD = 64
KD = 32
NPOS = 1600
EPS = 1e-5
SCALE = float(KD) ** -0.5
NMT = 13  # position tiles: 12*128 + 64
HALF = 800

_compiled_nc = None


def build_nc():
    import concourse.tile as tile
    from concourse import bacc, mybir

    f32 = mybir.dt.float32
    bf16 = mybir.dt.bfloat16
    fp8 = mybir.dt.float8e4
    DR = mybir.MatmulPerfMode.DoubleRow
    AF = mybir.ActivationFunctionType
    OP = mybir.AluOpType
    EXP_BIAS = -4.0  # keep exp(S+bias) within fp8e4m3 range; softmax-invariant

    nc = bacc.Bacc("TRN2", target_bir_lowering=False, debug=False, num_devices=8)

    x16_d = nc.dram_tensor("x16", [DIM, NPOS], bf16, kind="ExternalInput").ap()
    wq16_d = nc.dram_tensor("wq16", [DIM, 256], bf16, kind="ExternalInput").ap()
    wk16_d = nc.dram_tensor("wk16", [DIM, 256], bf16, kind="ExternalInput").ap()
    wv16_d = nc.dram_tensor("wv16", [DIM, DIM], bf16, kind="ExternalInput").ap()
    wp16_d = nc.dram_tensor("wp16", [DIM, DIM], bf16, kind="ExternalInput").ap()
    bq_d = nc.dram_tensor("bq", [128, 2], f32, kind="ExternalInput").ap()
    bk_d = nc.dram_tensor("bk", [128, 2], f32, kind="ExternalInput").ap()
    bv_d = nc.dram_tensor("bv", [128, 4], f32, kind="ExternalInput").ap()
    bvT_d = nc.dram_tensor("bvT", [1, DIM], bf16, kind="ExternalInput").ap()
    bpe_d = nc.dram_tensor("bpe", [128, 4], f32, kind="ExternalInput").ap()
    bp_d = nc.dram_tensor("bp", [128, 4], f32, kind="ExternalInput").ap()
    pdg_d = nc.dram_tensor("pdg", [36, 128, 128], bf16, kind="ExternalInput").ap()
    wpe_d = nc.dram_tensor("wpe", [128, 36], f32, kind="ExternalInput").ap()
    y_d = nc.dram_tensor("y", [DIM, NPOS], bf16, kind="ExternalOutput").ap()

    def mt_sz(j):
        return 64 if j == NMT - 1 else 128

    with tile.TileContext(nc) as tc:
        with (
            tc.tile_pool(name="pers", bufs=1) as pers,
            tc.tile_pool(name="ps2", bufs=2, space="PSUM") as ps2,
            tc.tile_pool(name="scp", bufs=2, space="PSUM") as scp,
            tc.tile_pool(name="mmp", bufs=1, space="PSUM") as mmp,
            tc.tile_pool(name="ep", bufs=4) as ep,
            tc.tile_pool(name="ystg", bufs=4) as ystg,
        ):
            # fp8 DoubleRow operands: [p, 2, n] — scores pad slot r=1 with zeros,
            # attn-out pairs adjacent m-tiles (2a, 2a+1) in the two slots.
            q_hi = [pers.tile([128, 2, NPOS], fp8, name=f"qhi{t}") for t in range(2)]
            k2 = [pers.tile([128, 2, NPOS], fp8, name=f"k2{t}") for t in range(2)]
            vT_sb = [
                pers.tile([128, 2, NH * 66], fp8, name=f"vT{a}") for a in range(7)
            ]
            E8 = [pers.tile([128, 2, HALF], fp8, name=f"E8{i}") for i in range(3)]
            z_sb = [pers.tile([128, NPOS], f32, name=f"z{t}") for t in range(4)]
            pe16 = [pers.tile([128, NPOS], bf16, name=f"pe{t}") for t in range(4)]
            pdg_sb = [pers.tile([128, 128], bf16, name=f"pdg{i}") for i in range(36)]
            wpe_sb = pers.tile([128, 36], f32, name="wpe_sb")
            bvT_bc = pers.tile([128, DIM], bf16, name="bvT_bc")
            s_g = [pers.tile([128, NPOS], f32, name=f"s_g{i}") for i in range(2)]
            r16 = [pers.tile([128, NPOS], bf16, name=f"r16{i}") for i in range(2)]
            wp_sb = [pers.tile([128, DIM], bf16, name=f"wp{c}") for c in range(4)]
            ones_sb = pers.tile([128, 512], bf16, name="ones_sb")
            x16_sb = [pers.tile([128, NPOS], bf16, name=f"x16{c}") for c in range(4)]
            wv_sb = [pers.tile([128, DIM], bf16, name=f"wv{c}") for c in range(4)]
            vpad = [pers.tile([128, 42 * 42], bf16, name=f"vpad{t}") for t in range(4)]
            wq_sb = [pers.tile([128, 256], bf16, name=f"wq{c}") for c in range(4)]
            wk_sb = [pers.tile([128, 256], bf16, name=f"wk{c}") for c in range(4)]
            bq_sb = pers.tile([128, 2], f32, name="bq_sb")
            bk_sb = pers.tile([128, 2], f32, name="bk_sb")
            bv_sb = pers.tile([128, 4], f32, name="bv_sb")
            bvT_sb = pers.tile([1, DIM], bf16, name="bvT_sb")
            bpe_sb = pers.tile([128, 4], f32, name="bpe_sb")
            bp_sb = pers.tile([128, 4], f32, name="bp_sb")
            ebias_sb = pers.tile([128, 1], f32, name="ebias_sb")
            z16 = [pers.tile([128, NPOS], bf16, name=f"z16{t}") for t in range(4)]

            nc.sync.dma_start(x16_sb[0][:], x16_d[0:128, :])
            for c in range(4):
                nc.sync.dma_start(wq_sb[c][:], wq16_d[128 * c : 128 * (c + 1), :])
                nc.sync.dma_start(wk_sb[c][:], wk16_d[128 * c : 128 * (c + 1), :])
            for c in range(1, 4):
                nc.sync.dma_start(x16_sb[c][:], x16_d[128 * c : 128 * (c + 1), :])
            for c in range(4):
                nc.sync.dma_start(wv_sb[c][:], wv16_d[128 * c : 128 * (c + 1), :])
                nc.sync.dma_start(wp_sb[c][:], wp16_d[128 * c : 128 * (c + 1), :])
            nc.sync.dma_start(bq_sb[:], bq_d[:])
            nc.sync.dma_start(bk_sb[:], bk_d[:])
            nc.sync.dma_start(bv_sb[:], bv_d[:])
            nc.sync.dma_start(bvT_sb[:], bvT_d[:])
            nc.sync.dma_start(bpe_sb[:], bpe_d[:])
            nc.sync.dma_start(bp_sb[:], bp_d[:])
            for i in range(36):
                nc.sync.dma_start(pdg_sb[i][:], pdg_d[i])
            nc.sync.dma_start(wpe_sb[:], wpe_d[:])
            nc.gpsimd.memset(ones_sb[:], 1.0)
            nc.gpsimd.memset(ebias_sb[:], EXP_BIAS)
            for t in range(4):
                nc.gpsimd.memset(vpad[t][:], 0.0)
            for t in range(2):
                nc.gpsimd.memset(q_hi[t][:, 1, :], 0.0)
                nc.gpsimd.memset(k2[t][:, 1, :], 0.0)
            for a in range(7):
                nc.gpsimd.memset(vT_sb[a][:], 0.0)
                vT_g = vT_sb[a].rearrange("p r (h g) -> p r h g", g=66)
                if a < 6:
                    nc.gpsimd.memset(vT_g[:, :, :, 64:65], 1.0)
                else:
                    nc.gpsimd.memset(vT_g[0:64, 0, :, 64:65], 1.0)

            # ---- stage A: q, k (4 heads x 32 rows per tile), v^T ----
            def qk_chain(w_sb, b_sb, dst, t, ch):
                cs = slice(400 * ch, 400 * (ch + 1))
                ps = ps2.tile([128, 512], f32, name="psqk", tag="ps2")
                for c in range(4):
                    nc.tensor.matmul(
                        ps[:, 0:400],
                        w_sb[c][:, 128 * t : 128 * (t + 1)],
                        x16_sb[c][:, cs],
                        start=(c == 0),
                        stop=(c == 3),
                    )
                nc.vector.tensor_scalar_add(
                    dst[t][:, 0, cs], ps[:, 0:400], b_sb[:, t : t + 1]
                )

            def vT_chain(j):
                mj = mt_sz(j)
                ps = ps2.tile([128, 512], f32, name="psvT", tag="ps2")
                for c in range(4):
                    nc.tensor.matmul(
                        ps[0:mj, :],
                        x16_sb[c][:, 128 * j : 128 * j + mj],
                        wv_sb[c][:],
                        start=(c == 0),
                        stop=(c == 3),
                    )
                vT_g = vT_sb[j // 2].rearrange("p r (h g) -> p r h g", g=66)
                nc.vector.tensor_tensor(
                    vT_g[0:mj, j % 2, :, 0:64],
                    ps[0:mj, :].rearrange("p (h d) -> p h d", d=64),
                    bvT_bc[0:mj, :].rearrange("p (h d) -> p h d", d=64),
                    op=OP.add,
                )

            # t=0 (heads 0-3) inline; t=1 (heads 4-7) deferred into the head loop
            for w_sb, b_sb, dst in ((wq_sb, bq_sb, q_hi), (wk_sb, bk_sb, k2)):
                for ch in range(4):
                    qk_chain(w_sb, b_sb, dst, 0, ch)
            ps = ps2.tile([128, 512], f32, name="psbc", tag="ps2")
            nc.tensor.matmul(ps[:, 0:512], ones_sb[0:1, 0:128], bvT_sb[0:1, :])
            nc.vector.tensor_copy(bvT_bc[:], ps[:, 0:512])
            for j in range(7):
                vT_chain(j)

            # ---- fillers: v-natural (PE) + depthwise-pe diag series ----
            def v_series(t, ch):
                cs = slice(400 * ch, 400 * (ch + 1))
                ps = ps2.tile([128, 512], f32, name="psv", tag="ps2")
                for c in range(4):
                    nc.tensor.matmul(
                        ps[:, 0:400],
                        wv_sb[c][:, 128 * t : 128 * (t + 1)],
                        x16_sb[c][:, cs],
                        start=(c == 0),
                        stop=(c == 3),
                    )
                dst = vpad[t].rearrange("p (a b) -> p a b", a=42)[
                    :, 1 + 10 * ch : 11 + 10 * ch, 1:41
                ]
                nc.vector.tensor_scalar_add(
                    dst,
                    ps[:, 0:400].rearrange("p (a b) -> p a b", a=10),
                    bv_sb[:, t : t + 1],
                )

            def pe_series(t, ch):
                # depthwise 3x3 as 9 diagonal bf16 matmuls (PE filler work)
                vg = vpad[t].rearrange("p (a b) -> p a b", a=42)
                ps = ps2.tile([128, 512], f32, name="pspe", tag="ps2")
                for k9 in range(9):
                    dy, dx = k9 // 3 - 1, k9 % 3 - 1
                    rhs = vg[:, 1 + 10 * ch + dy : 11 + 10 * ch + dy, 1 + dx : 41 + dx]
                    nc.tensor.matmul(
                        ps[:, 0:400],
                        pdg_sb[9 * t + k9][:],
                        rhs,
                        start=(k9 == 0),
                        stop=(k9 == 8),
                    )
                nc.vector.tensor_scalar_add(
                    pe16[t][:, 400 * ch : 400 * (ch + 1)],
                    ps[:, 0:400],
                    bpe_sb[:, t : t + 1],
                )

            def pe_dve(t):
                # depthwise 3x3 on DVE for image rows 20-39 (columns 800:1600)
                vg = vpad[t].rearrange("p (a b) -> p a b", a=42)
                dst = pe16[t][:, 800:1600].rearrange("p (a b) -> p a b", a=20)
                for k9 in range(9):
                    dy, dx = k9 // 3 - 1, k9 % 3 - 1
                    src = vg[:, 21 + dy : 41 + dy, 1 + dx : 41 + dx]
                    w_col = wpe_sb[:, 9 * t + k9 : 9 * t + k9 + 1]
                    if k9 == 0:
                        nc.vector.tensor_scalar(
                            dst, src, w_col, bpe_sb[:, t : t + 1],
                            op0=OP.mult, op1=OP.add,
                        )
                    else:
                        nc.vector.scalar_tensor_tensor(
                            dst, src, w_col, dst, op0=OP.mult, op1=OP.add
                        )

            # half-0 fillers: qk t=1, v-natural, pe chunks 0-1 (columns 0:800)
            jobs = []
            for w_sb, b_sb, dst in ((wq_sb, bq_sb, q_hi), (wk_sb, bk_sb, k2)):
                jobs += [("qk", (w_sb, b_sb, dst), ch) for ch in range(4)]
            for t in range(4):
                jobs += [("v", t, ch) for ch in range(4)]
                jobs += [("pe", t, 0), ("pe", t, 1)]
            # half-1 fillers: DVE dwconv for columns 800:1600, then proj of half 0
            jobs2 = [("peD", t, None) for t in range(4)]

            def run_job(job):
                kind, jt, jch = job
                if kind == "v":
                    v_series(jt, jch)
                elif kind == "qk":
                    qk_chain(jt[0], jt[1], jt[2], 1, jch)
                elif kind == "pe":
                    pe_series(jt, jch)
                elif kind == "peD":
                    pe_dve(jt)
                else:
                    proj_chain(*jt)

            def pop_jobs(q, n):
                for _ in range(n):
                    if q:
                        run_job(q.pop(0))

            # ---- per-group normalize (1/s broadcast) + pe-add ----
            def normalize_group(g, half, hs):
                nc.vector.reciprocal_approx_fast(s_g[g][:, hs], s_g[g][:, hs])
                nc.vector.tensor_copy(r16[g][:, hs], s_g[g][:, hs])
                c0 = slice(HALF * half, HALF * half + 512)
                c1 = slice(HALF * half + 512, HALF * half + 800)
                for t in (2 * g, 2 * g + 1):
                    for i2 in range(2):
                        sr = 32 * (2 * t + i2 - 4 * g)
                        rb = scp.tile([128, HALF], f32, name="rb", tag="sc")
                        for cc, ncols in ((c0, 512), (c1, 288)):
                            off = cc.start - HALF * half
                            nc.tensor.matmul(
                                rb[0:64, off : off + ncols],
                                ones_sb[sr : sr + 1, 0:64],
                                r16[g][sr : sr + 1, cc],
                                tile_position=(sr, 0),
                            )
                        rows = slice(64 * i2, 64 * i2 + 64)
                        nc.vector.tensor_tensor(
                            z_sb[t][rows, hs], z_sb[t][rows, hs], rb[0:64, :],
                            op=OP.mult,
                        )
                    nc.gpsimd.tensor_tensor(
                        z16[t][:, hs], z_sb[t][:, hs], pe16[t][:, hs], op=OP.add
                    )

            def proj_chain(half, o, ch):
                cs = slice(HALF * half + 400 * ch, HALF * half + 400 * (ch + 1))
                pj = ps2.tile([128, 512], f32, name="pj", tag="ps2")
                for c in range(4):
                    nc.tensor.matmul(
                        pj[:, 0:400],
                        wp_sb[c][:, 128 * o : 128 * (o + 1)],
                        z16[c][:, cs],
                        start=(c == 0),
                        stop=(c == 3),
                    )
                yt = ystg.tile([128, 400], bf16, name="yt", tag="yt")
                nc.vector.tensor_scalar_add(yt[:], pj[:, 0:400], bp_sb[:, o : o + 1])
                nc.sync.dma_start(y_d[128 * o : 128 * (o + 1), cs], yt[:])

            # ---- attention streaming + per-group assembly ----
            for half in range(2):
                hs = slice(HALF * half, HALF * (half + 1))
                c0 = slice(HALF * half, HALF * half + 512)
                c1 = slice(HALF * half + 512, HALF * half + 800)
                for h in range(8):
                    t = h // 4
                    i = h % 4
                    sr = 32 * i
                    mm = mmp.tile([65, HALF], f32, name="mm", tag="mm")

                    def mm3(a):
                        mja = 128 if a < 6 else 64
                        lhsT = vT_sb[a].rearrange("p r (h g) -> p r h g", g=66)[
                            0:mja, :, h, 0:65
                        ]
                        nc.tensor.matmul(
                            mm[:, 0:512],
                            lhsT,
                            E8[a % 3][0:mja, :, 0:512],
                            start=(a == 0),
                            stop=(a == 6),
                            perf_mode=DR,
                        )
                        nc.tensor.matmul(
                            mm[:, 512:800],
                            lhsT,
                            E8[a % 3][0:mja, :, 512:800],
                            start=(a == 0),
                            stop=(a == 6),
                            perf_mode=DR,
                        )

                    for j in range(NMT):
                        a, r = j // 2, j % 2
                        mj = mt_sz(j)
                        ms = slice(128 * j, 128 * j + mj)
                        sc = scp.tile([128, HALF], f32, name="sc", tag="sc")
                        nc.tensor.matmul(
                            sc[0:mj, 0:512],
                            k2[t][sr : sr + 32, :, ms],
                            q_hi[t][sr : sr + 32, :, c0],
                            tile_position=(sr, 0),
                            perf_mode=DR,
                        )
                        nc.tensor.matmul(
                            sc[0:mj, 512:800],
                            k2[t][sr : sr + 32, :, ms],
                            q_hi[t][sr : sr + 32, :, c1],
                            tile_position=(sr, 0),
                            perf_mode=DR,
                        )
                        if half == 0 and h == 0 and j < 6:
                            vT_chain(7 + j)
                        nc.scalar.activation(
                            E8[a % 3][0:mj, r, :],
                            sc[0:mj, :],
                            AF.Exp,
                            bias=ebias_sb[0:mj, 0:1],
                        )
                        if r == 1 and a >= 1:
                            mm3(a - 1)
                    mm3(5)
                    mm3(6)
                    rowbase = 64 * (h % 2)
                    nc.vector.tensor_copy(
                        z_sb[h // 2][rowbase : rowbase + 64, hs], mm[0:64, :]
                    )
                    nc.vector.tensor_copy(s_g[h // 4][sr : sr + 1, hs], mm[64:65, :])
                    if half == 0:
                        if h >= 1:
                            pop_jobs(jobs, 5)
                    else:
                        pop_jobs(jobs2, 2)
                    if h == 5:
                        normalize_group(0, half, hs)
                    if h == 7:
                        normalize_group(1, half, hs)
                        jobs2.extend(
                            ("proj", (half, o, ch), None)
                            for o in range(4)
                            for ch in range(2)
                        )
            while jobs2:
                run_job(jobs2.pop(0))
    nc.compile()
    return nc


def prep_weights(inputs):
    import ml_dtypes

    bfl = ml_dtypes.bfloat16
    d = lambda k: np.asarray(inputs[k], dtype=np.float64)
    inv = d("qkv_gamma") / np.sqrt(d("qkv_var") + EPS)
    W = d("qkv_w") * inv[:, None]
    bb = d("qkv_beta") - d("qkv_mean") * inv
    Wh = W.reshape(NH, 2 * KD + HD, DIM)
    bh = bb.reshape(NH, 2 * KD + HD)
    Wq = (Wh[:, :KD] * SCALE).reshape(NH * KD, DIM)
    bq = (bh[:, :KD] * SCALE).reshape(-1)
    Wk = Wh[:, KD : 2 * KD].reshape(NH * KD, DIM)
    bk = bh[:, KD : 2 * KD].reshape(-1)
    Wv = Wh[:, 2 * KD :].reshape(NH * HD, DIM)
    bv = bh[:, 2 * KD :].reshape(-1)

    ipe = d("pe_gamma") / np.sqrt(d("pe_var") + EPS)
    wpe = d("pe_w")[:, 0] * ipe[:, None, None]  # [512, 3, 3]
    bpe = d("pe_beta") - d("pe_mean") * ipe
    pdg = np.zeros((36, 128, 128), np.float64)
    wpe_cols = np.zeros((128, 36), np.float64)
    ar = np.arange(128)
    for t in range(4):
        for k9 in range(9):
            pdg[t * 9 + k9, ar, ar] = wpe[128 * t : 128 * (t + 1), k9 // 3, k9 % 3]
            wpe_cols[:, 9 * t + k9] = wpe[128 * t : 128 * (t + 1), k9 // 3, k9 % 3]

    ip = d("proj_gamma") / np.sqrt(d("proj_var") + EPS)
    Wp = d("proj_w") * ip[:, None]
    bp = d("proj_beta") - d("proj_mean") * ip

    c32 = lambda a: np.ascontiguousarray(a, dtype=np.float32)
    c16 = lambda a: np.ascontiguousarray(a.astype(np.float32), dtype=bfl)
    return dict(
        wq16=c16(Wq.T),
        wk16=c16(Wk.T),
        wv16=c16(Wv.T),
        wp16=c16(Wp.T),
        bq=c32(bq.reshape(2, 128).T),
        bk=c32(bk.reshape(2, 128).T),
        bv=c32(bv.reshape(4, 128).T),
        bvT=c16(bv[None]),
        bpe=c32(bpe.reshape(4, 128).T),
        bp=c32(bp.reshape(4, 128).T),
        pdg=c16(pdg),
        wpe=c32(wpe_cols),
    )


def make_in_maps(inputs):
    import ml_dtypes

    w = prep_weights(inputs)
    x = np.asarray(inputs["x"], dtype=np.float32)
    B = x.shape[0]
    maps = []
    for i in range(B):
        xi = np.ascontiguousarray(x[i].reshape(DIM, NPOS))
        maps.append({"x16": xi.astype(ml_dtypes.bfloat16), **w})
    return maps


def kernel(**inputs):
    global _compiled_nc
    from concourse.bass_utils import run_bass_kernel_spmd

    if _compiled_nc is None:
        _compiled_nc = build_nc()
    in_maps = make_in_maps(inputs)
    res = run_bass_kernel_spmd(_compiled_nc, in_maps, core_ids=list(range(8)))
    y = np.stack(
        [
            np.asarray(res.results[i]["y"], dtype=np.float32).reshape(DIM, 40, 40)
            for i in range(8)
        ]
    )
    return y


if __name__ == "__main__":
    nc = build_nc()
    print("built ok")


# revision 72
# speedup vs baseline: 1.1136x; 1.1136x over previous
"""Trainium2 Bass kernel for nn_Attention_56822417326562 (dense transformer block).

Sharding: data-parallel over batch — core i computes batch element i entirely
(B=8 over 8 NeuronCores, no collectives).

Per-core math (x: [512, 1600]):
  BN folded into weights on host; softmax scale folded into q.
  The whole attention path runs in fp8e4m3 with DoubleRow perf mode (2 fp8
  contraction rows per PE cell per cycle); safe because the attention output
  is ~7x smaller than the depthwise-conv branch it is summed with:
    - q/k/E/vT stored as [p, 2, n] fp8; scores pad the pair slot with zeros
      (contraction kd=32), attn-out pairs adjacent m-tiles (2a, 2a+1)
    - scores computed TRANSPOSED (S^T[m,n]=k.q) via PE row tiling
      (tile_position=(32i,0)), exp on ScalarE with a -4 bias folded in so
      exp(S-4) fits fp8 range (softmax-invariant)
    - out_un[d,n] and denominator s[n] in one PE accumulation via a ones
      column in v^T
  pe = depthwise 3x3: image rows 0-19 as 9 diagonal bf16 matmuls (PE filler
  that keeps the systolic array hot), rows 20-39 on DVE scalar_tensor_tensor.
  1/s applied via 1-term bf16 ones-matmul broadcast through PSUM.
  y = WpT.T @ ((out_un * (1/s)) + pe) + bias; bias added in the DVE
  PSUM->SBUF drain, output DMA'd as bf16.
  Work is scheduled as filler "jobs" inside the exp-bound attention slots
  (stage-A tail, v-natural, dwconv, prior-half projection) to keep the PE
  continuously busy without long dependency stalls.
"""
import sys

sys.path.insert(0, "/opt/trn_rl_repo")

import numpy as np

DIM = 512
NH = 8
HD = 64
KD = 32
NPOS = 1600
EPS = 1e-5
SCALE = float(KD) ** -0.5
NMT = 13  # position tiles: 12*128 + 64
HALF = 800

_compiled_nc = None


def build_nc():
    import concourse.tile as tile
    from concourse import bacc, mybir

    f32 = mybir.dt.float32
    bf16 = mybir.dt.bfloat16
    fp8 = mybir.dt.float8e4
    DR = mybir.MatmulPerfMode.DoubleRow
    AF = mybir.ActivationFunctionType
    OP = mybir.AluOpType
    EXP_BIAS = -4.0  # keep exp(S+bias) within fp8e4m3 range; softmax-invariant

    nc = bacc.Bacc("TRN2", target_bir_lowering=False, debug=False, num_devices=8)

    x16_d = nc.dram_tensor("x16", [DIM, NPOS], bf16, kind="ExternalInput").ap()
    wq16_d = nc.dram_tensor("wq16", [DIM, 256], bf16, kind="ExternalInput").ap()
    wk16_d = nc.dram_tensor("wk16", [DIM, 256], bf16, kind="ExternalInput").ap()
    wv16_d = nc.dram_tensor("wv16", [DIM, DIM], bf16, kind="ExternalInput").ap()
    wp16_d = nc.dram_tensor("wp16", [DIM, DIM], bf16, kind="ExternalInput").ap()
    bq_d = nc.dram_tensor("bq", [128, 2], f32, kind="ExternalInput").ap()
    bk_d = nc.dram_tensor("bk", [128, 2], f32, kind="ExternalInput").ap()
    bv_d = nc.dram_tensor("bv", [128, 4], f32, kind="ExternalInput").ap()
    bvT_d = nc.dram_tensor("bvT", [1, DIM], bf16, kind="ExternalInput").ap()
    bpe_d = nc.dram_tensor("bpe", [128, 4], f32, kind="ExternalInput").ap()
    bp_d = nc.dram_tensor("bp", [128, 4], f32, kind="ExternalInput").ap()
    pdg_d = nc.dram_tensor("pdg", [36, 128, 128], bf16, kind="ExternalInput").ap()
    wpe_d = nc.dram_tensor("wpe", [128, 36], f32, kind="ExternalInput").ap()
    y_d = nc.dram_tensor("y", [DIM, NPOS], bf16, kind="ExternalOutput").ap()

    def mt_sz(j):
        return 64 if j == NMT - 1 else 128

    with tile.TileContext(nc) as tc:
        with (
            tc.tile_pool(name="pers", bufs=1) as pers,
            tc.tile_pool(name="ps2", bufs=2, space="PSUM") as ps2,
            tc.tile_pool(name="scp", bufs=2, space="PSUM") as scp,
            tc.tile_pool(name="mmp", bufs=1, space="PSUM") as mmp,
            tc.tile_pool(name="ep", bufs=4) as ep,
            tc.tile_pool(name="ystg", bufs=4) as ystg,
        ):
            # fp8 DoubleRow operands: [p, 2, n] — scores pad slot r=1 with zeros,
            # attn-out pairs adjacent m-tiles (2a, 2a+1) in the two slots.
            q_hi = [pers.tile([128, 2, NPOS], fp8, name=f"qhi{t}") for t in range(2)]
            k2 = [pers.tile([128, 2, NPOS], fp8, name=f"k2{t}") for t in range(2)]
            vT_sb = [
                pers.tile([128, 2, NH * 66], fp8, name=f"vT{a}") for a in range(7)
            ]
            E8 = [pers.tile([128, 2, HALF], fp8, name=f"E8{i}") for i in range(3)]
            z_sb = [pers.tile([128, NPOS], f32, name=f"z{t}") for t in range(4)]
            pe16 = [pers.tile([128, NPOS], bf16, name=f"pe{t}") for t in range(4)]
            pdg_sb = [pers.tile([128, 128], bf16, name=f"pdg{i}") for i in range(36)]
            wpe_sb = pers.tile([128, 36], f32, name="wpe_sb")
            bvT_bc = pers.tile([128, DIM], bf16, name="bvT_bc")
            s_g = [pers.tile([128, NPOS], f32, name=f"s_g{i}") for i in range(2)]
            r16 = [pers.tile([128, NPOS], bf16, name=f"r16{i}") for i in range(2)]
            wp_sb = [pers.tile([128, DIM], bf16, name=f"wp{c}") for c in range(4)]
            ones_sb = pers.tile([128, 512], bf16, name="ones_sb")
            x16_sb = [pers.tile([128, NPOS], bf16, name=f"x16{c}") for c in range(4)]
            wv_sb = [pers.tile([128, DIM], bf16, name=f"wv{c}") for c in range(4)]
            vpad = [pers.tile([128, 42 * 42], bf16, name=f"vpad{t}") for t in range(4)]
            wq_sb = [pers.tile([128, 256], bf16, name=f"wq{c}") for c in range(4)]
            wk_sb = [pers.tile([128, 256], bf16, name=f"wk{c}") for c in range(4)]
            bq_sb = pers.tile([128, 2], f32, name="bq_sb")
            bk_sb = pers.tile([128, 2], f32, name="bk_sb")
            bv_sb = pers.tile([128, 4], f32, name="bv_sb")
            bvT_sb = pers.tile([1, DIM], bf16, name="bvT_sb")
            bpe_sb = pers.tile([128, 4], f32, name="bpe_sb")
            bp_sb = pers.tile([128, 4], f32, name="bp_sb")
            ebias_sb = pers.tile([128, 1], f32, name="ebias_sb")
            z16 = [pers.tile([128, NPOS], bf16, name=f"z16{t}") for t in range(4)]

            nc.sync.dma_start(x16_sb[0][:], x16_d[0:128, :])
            for c in range(4):
                nc.sync.dma_start(wq_sb[c][:], wq16_d[128 * c : 128 * (c + 1), :])
                nc.sync.dma_start(wk_sb[c][:], wk16_d[128 * c : 128 * (c + 1), :])
            for c in range(1, 4):
                nc.sync.dma_start(x16_sb[c][:], x16_d[128 * c : 128 * (c + 1), :])
            for c in range(4):
                nc.sync.dma_start(wv_sb[c][:], wv16_d[128 * c : 128 * (c + 1), :])
                nc.sync.dma_start(wp_sb[c][:], wp16_d[128 * c : 128 * (c + 1), :])
            nc.sync.dma_start(bq_sb[:], bq_d[:])
            nc.sync.dma_start(bk_sb[:], bk_d[:])
            nc.sync.dma_start(bv_sb[:], bv_d[:])
            nc.sync.dma_start(bvT_sb[:], bvT_d[:])
            nc.sync.dma_start(bpe_sb[:], bpe_d[:])
            nc.sync.dma_start(bp_sb[:], bp_d[:])
            for i in range(36):
                nc.sync.dma_start(pdg_sb[i][:], pdg_d[i])
            nc.sync.dma_start(wpe_sb[:], wpe_d[:])
            nc.gpsimd.memset(ones_sb[:], 1.0)
            nc.gpsimd.memset(ebias_sb[:], EXP_BIAS)
            for t in range(4):
                nc.gpsimd.memset(vpad[t][:], 0.0)
            for t in range(2):
                nc.gpsimd.memset(q_hi[t][:, 1, :], 0.0)
                nc.gpsimd.memset(k2[t][:, 1, :], 0.0)
            for a in range(7):
                nc.gpsimd.memset(vT_sb[a][:], 0.0)
                vT_g = vT_sb[a].rearrange("p r (h g) -> p r h g", g=66)
                if a < 6:
                    nc.gpsimd.memset(vT_g[:, :, :, 64:65], 1.0)
                else:
                    nc.gpsimd.memset(vT_g[0:64, 0, :, 64:65], 1.0)

            # ---- stage A: q, k (4 heads x 32 rows per tile), v^T ----
            def qk_chain(w_sb, b_sb, dst, t, ch):
                cs = slice(400 * ch, 400 * (ch + 1))
                ps = ps2.tile([128, 512], f32, name="psqk", tag="ps2")
                for c in range(4):
                    nc.tensor.matmul(
                        ps[:, 0:400],
                        w_sb[c][:, 128 * t : 128 * (t + 1)],
                        x16_sb[c][:, cs],
                        start=(c == 0),
                        stop=(c == 3),
                    )
                nc.vector.tensor_scalar_add(
                    dst[t][:, 0, cs], ps[:, 0:400], b_sb[:, t : t + 1]
                )

            def vT_chain(j):
                mj = mt_sz(j)
                ps = ps2.tile([128, 512], f32, name="psvT", tag="ps2")
                for c in range(4):
                    nc.tensor.matmul(
                        ps[0:mj, :],
                        x16_sb[c][:, 128 * j : 128 * j + mj],
                        wv_sb[c][:],
                        start=(c == 0),
                        stop=(c == 3),
                    )
                vT_g = vT_sb[j // 2].rearrange("p r (h g) -> p r h g", g=66)
                nc.vector.tensor_tensor(
                    vT_g[0:mj, j % 2, :, 0:64],
                    ps[0:mj, :].rearrange("p (h d) -> p h d", d=64),
                    bvT_bc[0:mj, :].rearrange("p (h d) -> p h d", d=64),
                    op=OP.add,
                )

            # t=0 (heads 0-3) inline; t=1 (heads 4-7) deferred into the head loop
            for w_sb, b_sb, dst in ((wq_sb, bq_sb, q_hi), (wk_sb, bk_sb, k2)):
                for ch in range(4):
                    qk_chain(w_sb, b_sb, dst, 0, ch)
            ps = ps2.tile([128, 512], f32, name="psbc", tag="ps2")
            nc.tensor.matmul(ps[:, 0:512], ones_sb[0:1, 0:128], bvT_sb[0:1, :])
            nc.vector.tensor_copy(bvT_bc[:], ps[:, 0:512])
            for j in range(7):
                vT_chain(j)

            # ---- fillers: v-natural (PE) + depthwise-pe diag series ----
            def v_series(t, ch):
                cs = slice(400 * ch, 400 * (ch + 1))
                ps = ps2.tile([128, 512], f32, name="psv", tag="ps2")
                for c in range(4):
                    nc.tensor.matmul(
                        ps[:, 0:400],
                        wv_sb[c][:, 128 * t : 128 * (t + 1)],
                        x16_sb[c][:, cs],
                        start=(c == 0),
                        stop=(c == 3),
                    )
                dst = vpad[t].rearrange("p (a b) -> p a b", a=42)[
                    :, 1 + 10 * ch : 11 + 10 * ch, 1:41
                ]
                nc.vector.tensor_scalar_add(
                    dst,
                    ps[:, 0:400].rearrange("p (a b) -> p a b", a=10),
                    bv_sb[:, t : t + 1],
                )

            def pe_series(t, ch):
                # depthwise 3x3 as 9 diagonal bf16 matmuls (PE filler work)
                vg = vpad[t].rearrange("p (a b) -> p a b", a=42)
                ps = ps2.tile([128, 512], f32, name="pspe", tag="ps2")
                for k9 in range(9):
                    dy, dx = k9 // 3 - 1, k9 % 3 - 1
                    rhs = vg[:, 1 + 10 * ch + dy : 11 + 10 * ch + dy, 1 + dx : 41 + dx]
                    nc.tensor.matmul(
                        ps[:, 0:400],
                        pdg_sb[9 * t + k9][:],
                        rhs,
                        start=(k9 == 0),
                        stop=(k9 == 8),
                    )
                nc.vector.tensor_scalar_add(
                    pe16[t][:, 400 * ch : 400 * (ch + 1)],
                    ps[:, 0:400],
                    bpe_sb[:, t : t + 1],
                )

            def pe_dve(t, seg):
                # depthwise 3x3 on DVE for image rows 20-39 (columns 800:1600),
                # 3 taps per job so PSUM drains never queue long behind it
                vg = vpad[t].rearrange("p (a b) -> p a b", a=42)
                dst = pe16[t][:, 800:1600].rearrange("p (a b) -> p a b", a=20)
                for k9 in range(3 * seg, 3 * seg + 3):
                    dy, dx = k9 // 3 - 1, k9 % 3 - 1
                    src = vg[:, 21 + dy : 41 + dy, 1 + dx : 41 + dx]
                    w_col = wpe_sb[:, 9 * t + k9 : 9 * t + k9 + 1]
                    if k9 == 0:
                        nc.vector.tensor_scalar(
                            dst, src, w_col, bpe_sb[:, t : t + 1],
                            op0=OP.mult, op1=OP.add,
                        )
                    else:
                        nc.vector.scalar_tensor_tensor(
                            dst, src, w_col, dst, op0=OP.mult, op1=OP.add
                        )

            # half-0 fillers: qk t=1, v-natural, pe chunks 0-1 (columns 0:800)
            jobs = []
            for w_sb, b_sb, dst in ((wq_sb, bq_sb, q_hi), (wk_sb, bk_sb, k2)):
                jobs += [("qk", (w_sb, b_sb, dst), ch) for ch in range(4)]
            for t in range(4):
                jobs += [("v", t, ch) for ch in range(4)]
                jobs += [("pe", t, 0), ("pe", t, 1)]
            # half-1 fillers: DVE dwconv for columns 800:1600, then proj of half 0
            jobs2 = [("peD", t, seg) for t in range(4) for seg in range(3)]

            def run_job(job):
                kind, jt, jch = job
                if kind == "v":
                    v_series(jt, jch)
                elif kind == "qk":
                    qk_chain(jt[0], jt[1], jt[2], 1, jch)
                elif kind == "pe":
                    pe_series(jt, jch)
                elif kind == "peD":
                    pe_dve(jt, jch)
                else:
                    proj_chain(*jt)

            def pop_jobs(q, n):
                for _ in range(n):
                    if q:
                        run_job(q.pop(0))

            # ---- per-group normalize (1/s broadcast) + pe-add ----
            def normalize_group(g, half, hs):
                nc.vector.reciprocal_approx_fast(s_g[g][:, hs], s_g[g][:, hs])
                nc.vector.tensor_copy(r16[g][:, hs], s_g[g][:, hs])
                c0 = slice(HALF * half, HALF * half + 512)
                c1 = slice(HALF * half + 512, HALF * half + 800)
                for t in (2 * g, 2 * g + 1):
                    for i2 in range(2):
                        sr = 32 * (2 * t + i2 - 4 * g)
                        rb = scp.tile([128, HALF], f32, name="rb", tag="sc")
                        for cc, ncols in ((c0, 512), (c1, 288)):
                            off = cc.start - HALF * half
                            nc.tensor.matmul(
                                rb[0:64, off : off + ncols],
                                ones_sb[sr : sr + 1, 0:64],
                                r16[g][sr : sr + 1, cc],
                                tile_position=(sr, 0),
                            )
                        rows = slice(64 * i2, 64 * i2 + 64)
                        nc.vector.tensor_tensor(
                            z_sb[t][rows, hs], z_sb[t][rows, hs], rb[0:64, :],
                            op=OP.mult,
                        )
                    nc.gpsimd.tensor_tensor(
                        z16[t][:, hs], z_sb[t][:, hs], pe16[t][:, hs], op=OP.add
                    )

            def proj_chain(half, o, ch):
                cs = slice(HALF * half + 400 * ch, HALF * half + 400 * (ch + 1))
                pj = ps2.tile([128, 512], f32, name="pj", tag="ps2")
                for c in range(4):
                    nc.tensor.matmul(
                        pj[:, 0:400],
                        wp_sb[c][:, 128 * o : 128 * (o + 1)],
                        z16[c][:, cs],
                        start=(c == 0),
                        stop=(c == 3),
                    )
                yt = ystg.tile([128, 400], bf16, name="yt", tag="yt")
                nc.vector.tensor_scalar_add(yt[:], pj[:, 0:400], bp_sb[:, o : o + 1])
                nc.sync.dma_start(y_d[128 * o : 128 * (o + 1), cs], yt[:])

            # ---- attention streaming + per-group assembly ----
            for half in range(2):
                hs = slice(HALF * half, HALF * (half + 1))
                c0 = slice(HALF * half, HALF * half + 512)
                c1 = slice(HALF * half + 512, HALF * half + 800)
                for h in range(8):
                    t = h // 4
                    i = h % 4
                    sr = 32 * i
                    mm = mmp.tile([65, HALF], f32, name="mm", tag="mm")

                    def mm3(a):
                        mja = 128 if a < 6 else 64
                        lhsT = vT_sb[a].rearrange("p r (h g) -> p r h g", g=66)[
                            0:mja, :, h, 0:65
                        ]
                        nc.tensor.matmul(
                            mm[:, 0:512],
                            lhsT,
                            E8[a % 3][0:mja, :, 0:512],
                            start=(a == 0),
                            stop=(a == 6),
                            perf_mode=DR,
                        )
                        nc.tensor.matmul(
                            mm[:, 512:800],
                            lhsT,
                            E8[a % 3][0:mja, :, 512:800],
                            start=(a == 0),
                            stop=(a == 6),
                            perf_mode=DR,
                        )

                    for j in range(NMT):
                        a, r = j // 2, j % 2
                        mj = mt_sz(j)
                        ms = slice(128 * j, 128 * j + mj)
                        sc = scp.tile([128, HALF], f32, name="sc", tag="sc")
                        nc.tensor.matmul(
                            sc[0:mj, 0:512],
                            k2[t][sr : sr + 32, :, ms],
                            q_hi[t][sr : sr + 32, :, c0],
                            tile_position=(sr, 0),
                            perf_mode=DR,
                        )
                        nc.tensor.matmul(
                            sc[0:mj, 512:800],
                            k2[t][sr : sr + 32, :, ms],
                            q_hi[t][sr : sr + 32, :, c1],
                            tile_position=(sr, 0),
                            perf_mode=DR,
                        )
                        if half == 0 and h == 0 and j < 6:
                            vT_chain(7 + j)
                        nc.scalar.activation(
                            E8[a % 3][0:mj, r, :],
                            sc[0:mj, :],
                            AF.Exp,
                            bias=ebias_sb[0:mj, 0:1],
                        )
                        if r == 1 and a >= 1:
                            mm3(a - 1)
                    mm3(5)
                    mm3(6)
                    rowbase = 64 * (h % 2)
                    # drains on ScalarE: DVE's in-order queue can be deep with
                    # dwconv taps; ScalarE frees the single mm accumulator fast
                    nc.scalar.copy(
                        z_sb[h // 2][rowbase : rowbase + 64, hs], mm[0:64, :]
                    )
                    nc.scalar.copy(s_g[h // 4][sr : sr + 1, hs], mm[64:65, :])
                    if half == 0:
                        if h >= 1:
                            pop_jobs(jobs, 5)
                    else:
                        pop_jobs(jobs2, 3)
                    if h == 5:
                        normalize_group(0, half, hs)
                    if h == 7:
                        normalize_group(1, half, hs)
                        jobs2.extend(
                            ("proj", (half, o, ch), None)
                            for o in range(4)
                            for ch in range(2)
                        )
            while jobs2:
                run_job(jobs2.pop(0))
    nc.compile()
    return nc


def prep_weights(inputs):
    import ml_dtypes

    bfl = ml_dtypes.bfloat16
    d = lambda k: np.asarray(inputs[k], dtype=np.float64)
    inv = d("qkv_gamma") / np.sqrt(d("qkv_var") + EPS)
    W = d("qkv_w") * inv[:, None]
    bb = d("qkv_beta") - d("qkv_mean") * inv
    Wh = W.reshape(NH, 2 * KD + HD, DIM)
    bh = bb.reshape(NH, 2 * KD + HD)
    Wq = (Wh[:, :KD] * SCALE).reshape(NH * KD, DIM)
    bq = (bh[:, :KD] * SCALE).reshape(-1)
    Wk = Wh[:, KD : 2 * KD].reshape(NH * KD, DIM)
    bk = bh[:, KD : 2 * KD].reshape(-1)
    Wv = Wh[:, 2 * KD :].reshape(NH * HD, DIM)
    bv = bh[:, 2 * KD :].reshape(-1)

    ipe = d("pe_gamma") / np.sqrt(d("pe_var") + EPS)
    wpe = d("pe_w")[:, 0] * ipe[:, None, None]  # [512, 3, 3]
    bpe = d("pe_beta") - d("pe_mean") * ipe
    pdg = np.zeros((36, 128, 128), np.float64)
    wpe_cols = np.zeros((128, 36), np.float64)
    ar = np.arange(128)
    for t in range(4):
        for k9 in range(9):
            pdg[t * 9 + k9, ar, ar] = wpe[128 * t : 128 * (t + 1), k9 // 3, k9 % 3]
            wpe_cols[:, 9 * t + k9] = wpe[128 * t : 128 * (t + 1), k9 // 3, k9 % 3]

    ip = d("proj_gamma") / np.sqrt(d("proj_var") + EPS)
    Wp = d("proj_w") * ip[:, None]
    bp = d("proj_beta") - d("proj_mean") * ip

    c32 = lambda a: np.ascontiguousarray(a, dtype=np.float32)
    c16 = lambda a: np.ascontiguousarray(a.astype(np.float32), dtype=bfl)
    return dict(
        wq16=c16(Wq.T),
        wk16=c16(Wk.T),
        wv16=c16(Wv.T),
        wp16=c16(Wp.T),
        bq=c32(bq.reshape(2, 128).T),
        bk=c32(bk.reshape(2, 128).T),
        bv=c32(bv.reshape(4, 128).T),
        bvT=c16(bv[None]),
        bpe=c32(bpe.reshape(4, 128).T),
        bp=c32(bp.reshape(4, 128).T),
        pdg=c16(pdg),
        wpe=c32(wpe_cols),
    )


def make_in_maps(inputs):
    import ml_dtypes

    w = prep_weights(inputs)
    x = np.asarray(inputs["x"], dtype=np.float32)
    B = x.shape[0]
    maps = []
    for i in range(B):
        xi = np.ascontiguousarray(x[i].reshape(DIM, NPOS))
        maps.append({"x16": xi.astype(ml_dtypes.bfloat16), **w})
    return maps


def kernel(**inputs):
    global _compiled_nc
    from concourse.bass_utils import run_bass_kernel_spmd

    if _compiled_nc is None:
        _compiled_nc = build_nc()
    in_maps = make_in_maps(inputs)
    res = run_bass_kernel_spmd(_compiled_nc, in_maps, core_ids=list(range(8)))
    y = np.stack(
        [
            np.asarray(res.results[i]["y"], dtype=np.float32).reshape(DIM, 40, 40)
            for i in range(8)
        ]
    )
    return y


if __name__ == "__main__":
    nc = build_nc()
    print("built ok")
